# revision 32
# baseline (speedup 1.0000x reference)
"""GNN message-passing node model on 8 TRN2 NeuronCores.

Reference computation:
    agg = segment_sum(edge_attr, edge_index[1], num_segments=N)   # scatter-add
    h   = relu(concat([x, agg], 1) @ W1 + b1)
    out = h @ W2 + b2

Sharding: destination nodes are split into 8 contiguous blocks of 6250;
edges are partitioned by destination (per the sharding hint), so the
scatter-add is fully local per core -- no halo exchange.

Per core the segment-sum runs as dense TensorE matmuls over 128-edge tiles:
    agg^T[128 feat, 128 nodes] += E_tile[128 edge, 128 feat]^T @ S_tile
where S_tile[e, n] = 1 iff edge e's window-relative destination is n.

Identity packing: the host places each edge at the PARTITION equal to its
window-relative destination whenever possible, so S is the constant
identity matrix for most tiles.  Each destination-node window of 128 nodes
gets cap_id identity tiles (a node with degree d fills slot r of the first
min(d, cap_id) identity tiles); edges beyond cap_id overflow into tiles
whose one-hot S is built on VectorE via is_equal(iota, colrel), batched
one DVE op per window.  cap_id is chosen per window to minimize total
tiles; window capacities are maxed over the 8 cores so every core runs one
identical SPMD program (padding slots carry zeros / colrel=200).

Edge features travel as fp8 e3m4 (4 mantissa bits, ~1.4e-2 rel overall vs
the 2e-2 gate): halves the dominant HBM stream, and FWL loads the fp8
stationary at 4B/cycle so LDWEIGHTS hides fully under each 128-col matmul.

Queues: edges ride the sync HWDGE queue, fully prefetched into SBUF (~81
KB/partition) with fine chunks for the first two groups so TensorE ramps
right after the ~7us runtime preamble; x rides the scalar HWDGE queue and
stays resident; output writes ride the gpsimd queue.  Consts are packed
into two arrays (one descriptor each).

The MLP is fused per 512-node group: h^T = relu(W1a^T x^T + W1b^T agg^T
+ b1) on ScalarE, out^T per 128-node window via W2 with the b2 bias
folded exactly into hT (c = W2^-T b2, a per-partition ScalarE add --
elided when b2 == 0), written in natural [node, feat] bf16 layout.
"""

import os
import sys
import types

import numpy as np
import ml_dtypes

N_NODES = 50000
N_EDGES = 600000
H = 128
N_CORES = 8
NPC = N_NODES // N_CORES          # 6250 nodes per core
WIN = 128                         # destination-node window (matmul N dim)
NW = (NPC + WIN - 1) // WIN       # 49 windows per core
NPAD = NW * WIN                   # 6272 padded nodes per core
GROUP = 4                         # windows per MLP group (512 nodes)
KMAX = 64                         # max identity-tile budget considered


def _install_axon_trace_shim():
    """If the harness sets BASS_TRACE=1, run_bass_kernel_spmd imports
    antenv.axon_hooks; slim axon containers lack it.  Provide the same
    ctypes-based NTFF hook trn_agent_boot would register, so tracing works
    instead of crashing.  No-op when the real module exists."""
    try:
        import antenv.axon_hooks  # noqa: F401
        return
    except ImportError:
        pass
    mod = types.ModuleType("antenv.axon_hooks")
    mod._hook = None
    mod.set_axon_ntff_profile_hook = lambda h: setattr(mod, "_hook", h)
    mod.get_axon_ntff_profile_hook = lambda: mod._hook
    sys.modules["antenv.axon_hooks"] = mod
    so_path = "/opt/axon/libaxon_pjrt.so"
    if os.path.exists(so_path):
        try:
            from trn_agent_boot.trn_boot import _ntff_profile_via_ctypes
            mod._hook = _ntff_profile_via_ctypes(so_path)
        except Exception:
            mod._hook = None
    try:
        from concourse import bass_utils
        _orig_upload = bass_utils.upload_artifacts

        def _safe_upload(tmpdir):
            try:
                return _orig_upload(tmpdir)
            except Exception as e:  # no bucket access in sandbox
                return f"upload-skipped({e.__class__.__name__})"

        bass_utils.upload_artifacts = _safe_upload
    except Exception:
        pass


def _prep_host(x, edge_index, edge_attr, W1, b1, W2, b2, np_edt):
    """Identity-pack edges per (core, window); build per-core input arrays."""
    bf16 = ml_dtypes.bfloat16
    col = np.asarray(edge_index)[1].astype(np.int64)
    core = col // NPC
    local = col - core * NPC               # 0..6249
    w = local // WIN                       # 0..48
    rel = local - w * WIN                  # 0..127

    # per-node degree and per-edge rank within its node
    deg = np.bincount(col, minlength=N_NODES)
    node_start = np.concatenate([[0], np.cumsum(deg)[:-1]])
    order = np.argsort(col, kind="stable")         # edges grouped by node
    scol = col[order]
    rank = np.arange(N_EDGES, dtype=np.int64) - node_start[scol]

    # choose cap_id per window: minimize cap_id + cap_ov (tiles per window)
    degs = np.zeros((N_CORES, NW * WIN), np.int64)
    degs[:, :NPC] = deg.reshape(N_CORES, NPC)
    degs = degs.reshape(N_CORES, NW, WIN)
    ks = np.arange(KMAX + 1)
    over = np.clip(degs[..., None] - ks, 0, None).sum(axis=2)   # [C, NW, K+1]
    over_max = over.max(axis=0)                                  # [NW, K+1]
    cap_ov_k = -(-over_max // WIN)
    cost = ks + cap_ov_k                                         # [NW, K+1]
    kstar = np.zeros(NW, np.int64)
    for wi in range(NW):
        c = cost[wi]
        best = int(np.min(c))
        kstar[wi] = int(np.max(np.nonzero(c == best)[0]))
    cap_id = kstar
    cap_ov = cap_ov_k[np.arange(NW), kstar]
    caps = np.maximum(1, cap_id + cap_ov)
    cap_id = np.minimum(cap_id, caps)              # keep >=1 total tile
    tstart = np.concatenate([[0], np.cumsum(caps)[:-1]])
    T = int(caps.sum())

    # slot assignment (in node-sorted edge order)
    e_core = core[order]
    e_w = w[order]
    e_rel = rel[order]
    is_id = rank < cap_id[e_w]
    tile_g = np.empty(N_EDGES, np.int64)
    part = np.empty(N_EDGES, np.int64)
    tile_g[is_id] = tstart[e_w[is_id]] + rank[is_id]
    part[is_id] = e_rel[is_id]
    # overflow edges: ordinal within (core, window) block
    ovm = ~is_id
    ovkey = e_core[ovm] * NW + e_w[ovm]
    ovcnt = np.bincount(ovkey, minlength=N_CORES * NW)
    ovstart = np.concatenate([[0], np.cumsum(ovcnt)[:-1]])
    ov_rank = np.arange(int(ovm.sum()), dtype=np.int64) - ovstart[ovkey]
    tile_g[ovm] = tstart[e_w[ovm]] + cap_id[e_w[ovm]] + ov_rank // WIN
    part[ovm] = ov_rank % WIN

    ea = np.asarray(edge_attr, np.float32).astype(np_edt)
    x = np.asarray(x, np.float32)
    W1 = np.asarray(W1, np.float32)
    b1 = np.asarray(b1, np.float32)
    W2 = np.asarray(W2, np.float32)
    b2 = np.asarray(b2, np.float32)

    iota = np.tile(np.arange(WIN, dtype=np.float32), (128, 1)).astype(bf16)
    ident = np.eye(128, dtype=np.float32).astype(bf16)
    w1a = np.ascontiguousarray(W1[:H]).astype(bf16)
    w1b = np.ascontiguousarray(W1[H:]).astype(bf16)
    w2c = W2.astype(bf16)
    b1c = np.ascontiguousarray(b1.reshape(H, 1))            # f32, ACT bias
    # fold b2 into hT: out = hT.T@W2 + b2 == (hT + c 1^T).T @ W2 with
    # c = W2^-T b2 (exact; c == 0 when b2 == 0, letting the add be elided)
    try:
        cvec = np.linalg.solve(W2.T.astype(np.float64),
                               b2.astype(np.float64)).astype(np.float32)
    except np.linalg.LinAlgError:
        cvec = np.linalg.lstsq(W2.T.astype(np.float64),
                               b2.astype(np.float64), rcond=None)[0].astype(
                                   np.float32)
    c_zero = bool(np.all(b2 == 0.0)) or bool(np.all(cvec == 0.0))
    cbf = np.concatenate([ident, iota, w1a, w1b, w2c], axis=1)

    # overflow tiles: global tile index -> compact overflow column index
    ov_tiles = []
    for wi in range(NW):
        for t in range(int(cap_id[wi]), int(caps[wi])):
            ov_tiles.append(int(tstart[wi]) + t)
    NOV = max(1, len(ov_tiles))
    ov_col = np.full(T, -1, np.int64)
    for jj, tg in enumerate(ov_tiles):
        ov_col[tg] = jj

    in_maps = []
    for c in range(N_CORES):
        m = (e_core == c)
        edges_c = np.zeros((128, T, H), dtype=np_edt)
        edges_c[part[m], tile_g[m], :] = ea[order[m]]
        colrel_c = np.full((128, NOV), 200.0, np.float32)
        mo = m & ovm
        colrel_c[part[mo], ov_col[tile_g[mo]]] = e_rel[mo]
        cf32_c = np.concatenate([colrel_c, b1c, cvec.reshape(H, 1)], axis=1)
        xT_c = np.zeros((H, NPAD), dtype=bf16)
        xT_c[:, :NPC] = x[c * NPC:(c + 1) * NPC].T.astype(bf16)
        in_maps.append({
            "edges": edges_c,
            "cf32": cf32_c, "xT": xT_c, "cbf": cbf,
        })
    return (in_maps, cap_id.tolist(), caps.tolist(), tstart.tolist(), T,
            NOV, c_zero)


def _build_program(cap_id, caps, tstart, T, NOV, c_zero, e_dt):
    import concourse.tile as tile
    from concourse import bacc, mybir
    from contextlib import ExitStack

    f32 = mybir.dt.float32
    bf16 = mybir.dt.bfloat16
    nc = bacc.Bacc("TRN2", target_bir_lowering=False, debug=False,
                   num_devices=N_CORES)

    CBF_COLS = 5 * 128
    edges_ap = nc.dram_tensor("edges", [128, T, H], e_dt, kind="ExternalInput").ap()
    cf32_ap = nc.dram_tensor("cf32", [128, NOV + 2], f32, kind="ExternalInput").ap()
    xT_ap = nc.dram_tensor("xT", [H, NPAD], bf16, kind="ExternalInput").ap()
    cbf_ap = nc.dram_tensor("cbf", [128, CBF_COLS], bf16, kind="ExternalInput").ap()
    out_dt = bf16  # bf16 store halves output DMA; host casts back to f32
    out_ap = nc.dram_tensor("out", [NPC, H], out_dt, kind="ExternalOutput").ap()

    n_groups = (NW + GROUP - 1) // GROUP
    ov_start = [0] * NW
    acc = 0
    for wi in range(NW):
        ov_start[wi] = acc
        acc += caps[wi] - cap_id[wi]

    with tile.TileContext(nc) as tc, ExitStack() as ctx:
        const = ctx.enter_context(tc.tile_pool(name="const", bufs=1))
        epool = ctx.enter_context(tc.tile_pool(name="edges", bufs=n_groups))
        spool = ctx.enter_context(tc.tile_pool(name="sel", bufs=8))
        aggp = ctx.enter_context(tc.tile_pool(name="agg", bufs=1))
        hpool = ctx.enter_context(tc.tile_pool(name="h", bufs=2))
        opool = ctx.enter_context(tc.tile_pool(name="osb", bufs=2))
        pw = ctx.enter_context(tc.tile_pool(name="pw", bufs=3, space="PSUM"))
        ph = ctx.enter_context(tc.tile_pool(name="ph", bufs=2, space="PSUM"))
        po = ctx.enter_context(tc.tile_pool(name="po", bufs=2, space="PSUM"))

        # consts: two packed DMAs; cbf (ident etc.) leads the sync queue,
        # cf32 (colrel, needed a bit later) is interleaved after the first
        # edge chunk below
        cbf_t = const.tile([128, CBF_COLS], bf16)
        nc.sync.dma_start(cbf_t[:], cbf_ap[:])
        cf32_t = const.tile([128, NOV + 2], f32)
        ident_t = cbf_t[:, 0:128]
        iota_t = cbf_t[:, 128:256]
        w1a_t = cbf_t[:, 256:384]
        w1b_t = cbf_t[:, 384:512]
        w2_t = cbf_t[:, 512:640]
        colrel_t = cf32_t[:, :NOV]
        b1_t = cf32_t[:, NOV:NOV + 1]
        c_t = cf32_t[:, NOV + 1:NOV + 2]
        # x on the gpsimd queue (its only traffic); stays resident in SBUF
        xT_t = const.tile([H, NPAD], bf16)
        nc.gpsimd.dma_start(xT_t[:, :GROUP * WIN], xT_ap[:, :GROUP * WIN])
        nc.gpsimd.dma_start(xT_t[:, GROUP * WIN:], xT_ap[:, GROUP * WIN:])

        aggT = aggp.tile([H, NPAD], bf16)

        # Full-prefetch edge DMAs (everything fits in SBUF), alternating
        # between the two HWDGE queues (sync / scalar) so per-trigger
        # engine time (~0.7us each) doesn't pace the stream.  Group 0 is
        # split [2 tiles | rest] so TensorE starts right after the runtime
        # preamble; cf32 (colrel) rides after the first chunk.
        gmeta = []
        etiles = []
        for g in range(n_groups):
            wlist = list(range(g * GROUP, min((g + 1) * GROUP, NW)))
            gt0 = tstart[wlist[0]]
            gtiles = sum(caps[w] for w in wlist)
            gmeta.append((wlist, gt0, gtiles))
            etile = epool.tile([128, gtiles * H], e_dt, tag="edges")
            etiles.append(etile)
            q = nc.sync if g % 2 == 0 else nc.scalar
            if g == 0:
                c1 = min(2, caps[wlist[0]])
                nc.sync.dma_start(
                    etile[:, :c1 * H],
                    edges_ap[:, gt0:gt0 + c1, :].rearrange("p t h -> p (t h)"))
                nc.sync.dma_start(cf32_t[:], cf32_ap[:])
                nc.sync.dma_start(
                    etile[:, c1 * H:],
                    edges_ap[:, gt0 + c1:gt0 + gtiles, :].rearrange(
                        "p t h -> p (t h)"))
            else:
                q.dma_start(
                    etile[:],
                    edges_ap[:, gt0:gt0 + gtiles, :].rearrange(
                        "p t h -> p (t h)"))

        for g in range(n_groups):
            wlist, gt0, gtiles = gmeta[g]
            nwin = len(wlist)
            etile = etiles[g]
            c0 = g * GROUP * WIN
            ncols = nwin * WIN
            xg = xT_t[:, c0:c0 + ncols]

            pwg = pw.tile([H, nwin * WIN], f32, tag="pw")
            for wi, w in enumerate(wlist):
                t0 = tstart[w] - gt0
                pslice = pwg[:, wi * WIN:(wi + 1) * WIN]
                kov = caps[w] - cap_id[w]
                Sb = None
                if kov:
                    Sb = spool.tile([128, kov * WIN], bf16, tag="S")
                    i0 = ov_start[w]
                    nc.vector.scalar_tensor_tensor(
                        out=Sb[:].rearrange("p (k q) -> p k q", k=kov),
                        in0=iota_t.unsqueeze(1).to_broadcast([128, kov, WIN]),
                        scalar=1.0,
                        in1=colrel_t[:, i0:i0 + kov].unsqueeze(2).to_broadcast(
                            [128, kov, WIN]),
                        op0=mybir.AluOpType.bypass,
                        op1=mybir.AluOpType.is_equal)
                for t in range(caps[w]):
                    lhsT = etile[:, (t0 + t) * H:(t0 + t + 1) * H]
                    if t < cap_id[w]:
                        rhs = ident_t
                    else:
                        ko = t - cap_id[w]
                        rhs = Sb[:, ko * WIN:(ko + 1) * WIN]
                    nc.tensor.matmul(out=pslice, lhsT=lhsT, rhs=rhs,
                                     start=(t == 0), stop=(t == caps[w] - 1))
            nc.vector.tensor_copy(aggT[:, c0:c0 + ncols], pwg[:])

            phh = ph.tile([H, ncols], f32, tag="ph")
            nc.tensor.matmul(phh[:], lhsT=w1a_t, rhs=xg,
                             start=True, stop=False)
            nc.tensor.matmul(phh[:], lhsT=w1b_t, rhs=aggT[:, c0:c0 + ncols],
                             start=False, stop=True)
            hT = hpool.tile([H, ncols], bf16, tag="hT")
            nc.scalar.activation(out=hT[:], in_=phh[:],
                                 func=mybir.ActivationFunctionType.Relu,
                                 bias=b1_t, scale=1.0)
            if not c_zero:
                # per-feature b2 fold: hT += c (c = W2^-T b2, per partition)
                nc.scalar.add(hT[:], hT[:], c_t)
            pog = po.tile([128, ncols], f32, tag="po")
            for k in range(nwin):
                nc.tensor.matmul(pog[:, k * WIN:(k + 1) * WIN],
                                 lhsT=hT[:, k * WIN:(k + 1) * WIN],
                                 rhs=w2_t, start=(k == 0),
                                 stop=(k == nwin - 1))
            osb = opool.tile([128, ncols], out_dt, tag="osb")
            nc.scalar.copy(osb[:], pog[:])

            rows = min(NPC - c0, ncols)
            # out write on the scalar HWDGE queue: the trigger directly
            # follows the osb copy that feeds it, so its semaphore wait is
            # already satisfied and never blocks later scalar work
            if rows == ncols:
                nc.scalar.dma_start(
                    out_ap[c0:c0 + ncols, :].rearrange("(t p) h -> p t h", p=128),
                    osb[:].rearrange("p (t h) -> p t h", t=nwin))
            else:
                full = rows // WIN
                for k in range(full):
                    nc.scalar.dma_start(out_ap[c0 + k * WIN:c0 + (k + 1) * WIN, :],
                                        osb[:, k * WIN:(k + 1) * WIN])
                rem = rows - full * WIN
                if rem:
                    nc.scalar.dma_start(
                        out_ap[c0 + full * WIN:c0 + rows, :],
                        osb[:rem, full * WIN:(full + 1) * WIN])

    nc.finalize()
    return nc


def kernel(x, edge_index, edge_attr, u=None, batch=None, W1=None, b1=None,
           W2=None, b2=None, **_unused):
    _install_axon_trace_shim()
    from concourse import mybir
    from concourse.bass_utils import run_bass_kernel_spmd

    np_edt, e_dt = ml_dtypes.float8_e3m4, mybir.dt.float8e3

    in_maps, cap_id, caps, tstart, T, NOV, c_zero = _prep_host(
        x, edge_index, edge_attr, W1, b1, W2, b2, np_edt)
    nc = _build_program(cap_id, caps, tstart, T, NOV, c_zero, e_dt)
    res = run_bass_kernel_spmd(nc, in_maps, core_ids=list(range(N_CORES)))
    out = np.concatenate(
        [np.asarray(res.results[c]["out"], np.float32) for c in range(N_CORES)],
        axis=0)
    return np.ascontiguousarray(out, dtype=np.float32)


# revision 42
# speedup vs baseline: 1.1011x; 1.1011x over previous
"""GNN message-passing node model on 8 TRN2 NeuronCores.

Reference computation:
    agg = segment_sum(edge_attr, edge_index[1], num_segments=N)   # scatter-add
    h   = relu(concat([x, agg], 1) @ W1 + b1)
    out = h @ W2 + b2

Sharding: destination nodes are split into 8 contiguous blocks of 6250;
edges are partitioned by destination (per the sharding hint), so the
scatter-add is fully local per core -- no halo exchange.

Per core the segment-sum runs as dense TensorE matmuls over 128-edge tiles:
    agg^T[128 feat, 128 nodes] += E_tile[128 edge, 128 feat]^T @ S_tile
where S_tile[e, n] = 1 iff edge e's window-relative destination is n.

Identity packing: the host places each edge at the PARTITION equal to its
window-relative destination whenever possible, so S is the constant
identity matrix for most tiles.  Each destination-node window of 128 nodes
gets cap_id identity tiles (a node with degree d fills slot r of the first
min(d, cap_id) identity tiles); edges beyond cap_id overflow into tiles
whose one-hot S is built on VectorE via is_equal(iota, colrel), batched
one DVE op per window.  cap_id is chosen per window to minimize total
tiles; window capacities are maxed over the 8 cores so every core runs one
identical SPMD program (padding slots carry zeros / colrel=200).

Edge features travel as fp8 e3m4 (4 mantissa bits, ~1.4e-2 rel overall vs
the 2e-2 gate): halves the dominant HBM stream, and FWL loads the fp8
stationary at 4B/cycle so LDWEIGHTS hides fully under each 128-col matmul.

Queues: edges ride the sync HWDGE queue, fully prefetched into SBUF (~81
KB/partition) with fine chunks for the first two groups so TensorE ramps
right after the ~7us runtime preamble; x rides the scalar HWDGE queue and
stays resident; output writes ride the gpsimd queue.  Consts are packed
into two arrays (one descriptor each).

The MLP is fused per 512-node group: h^T = relu(W1a^T x^T + W1b^T agg^T
+ b1) on ScalarE, out^T per 128-node window via W2 with the b2 bias
folded exactly into hT (c = W2^-T b2, a per-partition ScalarE add --
elided when b2 == 0), written in natural [node, feat] bf16 layout.
"""

import os
import sys
import types

import numpy as np
import ml_dtypes

N_NODES = 50000
N_EDGES = 600000
H = 128
N_CORES = 8
NPC = N_NODES // N_CORES          # 6250 nodes per core
WIN = 128                         # destination-node window (matmul N dim)
NW = (NPC + WIN - 1) // WIN       # 49 windows per core
NPAD = NW * WIN                   # 6272 padded nodes per core
GROUP = 4                         # windows per MLP group (512 nodes)
KMAX = 64                         # max identity-tile budget considered


def _install_axon_trace_shim():
    """If the harness sets BASS_TRACE=1, run_bass_kernel_spmd imports
    antenv.axon_hooks; slim axon containers lack it.  Provide the same
    ctypes-based NTFF hook trn_agent_boot would register, so tracing works
    instead of crashing.  No-op when the real module exists."""
    try:
        import antenv.axon_hooks  # noqa: F401
        return
    except ImportError:
        pass
    mod = types.ModuleType("antenv.axon_hooks")
    mod._hook = None
    mod.set_axon_ntff_profile_hook = lambda h: setattr(mod, "_hook", h)
    mod.get_axon_ntff_profile_hook = lambda: mod._hook
    sys.modules["antenv.axon_hooks"] = mod
    so_path = "/opt/axon/libaxon_pjrt.so"
    if os.path.exists(so_path):
        try:
            from trn_agent_boot.trn_boot import _ntff_profile_via_ctypes
            mod._hook = _ntff_profile_via_ctypes(so_path)
        except Exception:
            mod._hook = None
    try:
        from concourse import bass_utils
        _orig_upload = bass_utils.upload_artifacts

        def _safe_upload(tmpdir):
            try:
                return _orig_upload(tmpdir)
            except Exception as e:  # no bucket access in sandbox
                return f"upload-skipped({e.__class__.__name__})"

        bass_utils.upload_artifacts = _safe_upload
    except Exception:
        pass


def _prep_host(x, edge_index, edge_attr, W1, b1, W2, b2, np_edt):
    """Identity-pack edges per (core, window); build per-core input arrays."""
    bf16 = ml_dtypes.bfloat16
    col = np.asarray(edge_index)[1].astype(np.int64)
    core = col // NPC
    local = col - core * NPC               # 0..6249
    w = local // WIN                       # 0..48
    rel = local - w * WIN                  # 0..127

    # per-node degree and per-edge rank within its node
    deg = np.bincount(col, minlength=N_NODES)
    node_start = np.concatenate([[0], np.cumsum(deg)[:-1]])
    order = np.argsort(col, kind="stable")         # edges grouped by node
    scol = col[order]
    rank = np.arange(N_EDGES, dtype=np.int64) - node_start[scol]

    # choose cap_id per window: minimize cap_id + cap_ov (tiles per window)
    degs = np.zeros((N_CORES, NW * WIN), np.int64)
    degs[:, :NPC] = deg.reshape(N_CORES, NPC)
    degs = degs.reshape(N_CORES, NW, WIN)
    ks = np.arange(KMAX + 1)
    over = np.clip(degs[..., None] - ks, 0, None).sum(axis=2)   # [C, NW, K+1]
    over_max = over.max(axis=0)                                  # [NW, K+1]
    cap_ov_k = -(-over_max // WIN)
    cost = ks + cap_ov_k                                         # [NW, K+1]
    kstar = np.zeros(NW, np.int64)
    for wi in range(NW):
        c = cost[wi]
        best = int(np.min(c))
        kstar[wi] = int(np.max(np.nonzero(c == best)[0]))
    cap_id = kstar
    cap_ov = cap_ov_k[np.arange(NW), kstar]
    caps = np.maximum(1, cap_id + cap_ov)
    cap_id = np.minimum(cap_id, caps)              # keep >=1 total tile
    tstart = np.concatenate([[0], np.cumsum(caps)[:-1]])
    T = int(caps.sum())

    # slot assignment (in node-sorted edge order)
    e_core = core[order]
    e_w = w[order]
    e_rel = rel[order]
    is_id = rank < cap_id[e_w]
    tile_g = np.empty(N_EDGES, np.int64)
    part = np.empty(N_EDGES, np.int64)
    tile_g[is_id] = tstart[e_w[is_id]] + rank[is_id]
    part[is_id] = e_rel[is_id]
    # overflow edges: ordinal within (core, window) block
    ovm = ~is_id
    ovkey = e_core[ovm] * NW + e_w[ovm]
    ovcnt = np.bincount(ovkey, minlength=N_CORES * NW)
    ovstart = np.concatenate([[0], np.cumsum(ovcnt)[:-1]])
    ov_rank = np.arange(int(ovm.sum()), dtype=np.int64) - ovstart[ovkey]
    tile_g[ovm] = tstart[e_w[ovm]] + cap_id[e_w[ovm]] + ov_rank // WIN
    part[ovm] = ov_rank % WIN

    ea = np.asarray(edge_attr, np.float32).astype(np_edt)
    x = np.asarray(x, np.float32)
    W1 = np.asarray(W1, np.float32)
    b1 = np.asarray(b1, np.float32)
    W2 = np.asarray(W2, np.float32)
    b2 = np.asarray(b2, np.float32)

    iota = np.tile(np.arange(WIN, dtype=np.float32), (128, 1)).astype(bf16)
    ident = np.eye(128, dtype=np.float32).astype(bf16)
    w1a = np.ascontiguousarray(W1[:H]).astype(bf16)
    w1b = np.ascontiguousarray(W1[H:]).astype(bf16)
    w2c = W2.astype(bf16)
    b1c = np.ascontiguousarray(b1.reshape(H, 1))            # f32, ACT bias
    # fold b2 into hT: out = hT.T@W2 + b2 == (hT + c 1^T).T @ W2 with
    # c = W2^-T b2 (exact; c == 0 when b2 == 0, letting the add be elided)
    try:
        cvec = np.linalg.solve(W2.T.astype(np.float64),
                               b2.astype(np.float64)).astype(np.float32)
    except np.linalg.LinAlgError:
        cvec = np.linalg.lstsq(W2.T.astype(np.float64),
                               b2.astype(np.float64), rcond=None)[0].astype(
                                   np.float32)
    c_zero = bool(np.all(b2 == 0.0)) or bool(np.all(cvec == 0.0))
    cbf = np.concatenate([ident, iota, w1a, w1b, w2c], axis=1)

    # overflow tiles: global tile index -> compact overflow column index
    ov_tiles = []
    for wi in range(NW):
        for t in range(int(cap_id[wi]), int(caps[wi])):
            ov_tiles.append(int(tstart[wi]) + t)
    NOV = max(1, len(ov_tiles))
    ov_col = np.full(T, -1, np.int64)
    for jj, tg in enumerate(ov_tiles):
        ov_col[tg] = jj

    in_maps = []
    for c in range(N_CORES):
        m = (e_core == c)
        edges_c = np.zeros((128, T, H), dtype=np_edt)
        edges_c[part[m], tile_g[m], :] = ea[order[m]]
        colrel_c = np.full((128, NOV), 200.0, np.float32)
        mo = m & ovm
        colrel_c[part[mo], ov_col[tile_g[mo]]] = e_rel[mo]
        cf32_c = np.concatenate([colrel_c, b1c, cvec.reshape(H, 1)], axis=1)
        xT_c = np.zeros((H, NPAD), dtype=bf16)
        xT_c[:, :NPC] = x[c * NPC:(c + 1) * NPC].T.astype(bf16)
        in_maps.append({
            "edges": edges_c,
            "cf32": cf32_c, "xT": xT_c, "cbf": cbf,
        })
    return (in_maps, cap_id.tolist(), caps.tolist(), tstart.tolist(), T,
            NOV, c_zero)


def _build_program(cap_id, caps, tstart, T, NOV, c_zero, e_dt):
    import concourse.tile as tile
    from concourse import bacc, mybir
    from contextlib import ExitStack

    f32 = mybir.dt.float32
    bf16 = mybir.dt.bfloat16
    nc = bacc.Bacc("TRN2", target_bir_lowering=False, debug=False,
                   num_devices=N_CORES)

    CBF_COLS = 5 * 128
    edges_ap = nc.dram_tensor("edges", [128, T, H], e_dt, kind="ExternalInput").ap()
    cf32_ap = nc.dram_tensor("cf32", [128, NOV + 2], f32, kind="ExternalInput").ap()
    xT_ap = nc.dram_tensor("xT", [H, NPAD], bf16, kind="ExternalInput").ap()
    cbf_ap = nc.dram_tensor("cbf", [128, CBF_COLS], bf16, kind="ExternalInput").ap()
    out_dt = bf16  # bf16 store halves output DMA; host casts back to f32
    out_ap = nc.dram_tensor("out", [NPC, H], out_dt, kind="ExternalOutput").ap()

    n_groups = (NW + GROUP - 1) // GROUP
    ov_start = [0] * NW
    acc = 0
    for wi in range(NW):
        ov_start[wi] = acc
        acc += caps[wi] - cap_id[wi]

    with tile.TileContext(nc) as tc, ExitStack() as ctx:
        const = ctx.enter_context(tc.tile_pool(name="const", bufs=1))
        epool = ctx.enter_context(tc.tile_pool(name="edges", bufs=n_groups))
        spool = ctx.enter_context(tc.tile_pool(name="sel", bufs=8))
        aggp = ctx.enter_context(tc.tile_pool(name="agg", bufs=1))
        hpool = ctx.enter_context(tc.tile_pool(name="h", bufs=2))
        opool = ctx.enter_context(tc.tile_pool(name="osb", bufs=2))
        pw = ctx.enter_context(tc.tile_pool(name="pw", bufs=3, space="PSUM"))
        ph = ctx.enter_context(tc.tile_pool(name="ph", bufs=2, space="PSUM"))
        po = ctx.enter_context(tc.tile_pool(name="po", bufs=2, space="PSUM"))

        # consts: two packed DMAs lead the sync queue
        cbf_t = const.tile([128, CBF_COLS], bf16)
        nc.sync.dma_start(cbf_t[:], cbf_ap[:])
        cf32_t = const.tile([128, NOV + 2], f32)
        nc.sync.dma_start(cf32_t[:], cf32_ap[:])
        ident_t = cbf_t[:, 0:128]
        iota_t = cbf_t[:, 128:256]
        w1a_t = cbf_t[:, 256:384]
        w1b_t = cbf_t[:, 384:512]
        w2_t = cbf_t[:, 512:640]
        colrel_t = cf32_t[:, :NOV]
        b1_t = cf32_t[:, NOV:NOV + 1]
        c_t = cf32_t[:, NOV + 1:NOV + 2]
        # x on the scalar HWDGE queue; stays resident in SBUF all kernel
        xT_t = const.tile([H, NPAD], bf16)
        nc.scalar.dma_start(xT_t[:, :GROUP * WIN], xT_ap[:, :GROUP * WIN])
        nc.scalar.dma_start(xT_t[:, GROUP * WIN:], xT_ap[:, GROUP * WIN:])

        aggT = aggp.tile([H, NPAD], bf16)

        # Full-prefetch edge DMAs on the sync queue (everything fits in
        # SBUF).  Fine per-window chunks for the first two groups so
        # TensorE ramps right after the runtime preamble; coarse per-group
        # chunks after that to keep trigger count low.
        gmeta = []
        etiles = []
        for g in range(n_groups):
            wlist = list(range(g * GROUP, min((g + 1) * GROUP, NW)))
            gt0 = tstart[wlist[0]]
            gtiles = sum(caps[w] for w in wlist)
            gmeta.append((wlist, gt0, gtiles))
            etile = epool.tile([128, gtiles * H], e_dt, tag="edges")
            etiles.append(etile)
            if g < 2:
                off = 0
                for wi, w in enumerate(wlist):
                    t0 = tstart[w]
                    seg = caps[w] * H
                    if g == 0 and wi == 0:
                        c1 = min(2, caps[w])
                        nc.sync.dma_start(
                            etile[:, :c1 * H],
                            edges_ap[:, t0:t0 + c1, :].rearrange(
                                "p t h -> p (t h)"))
                        if caps[w] > c1:
                            nc.sync.dma_start(
                                etile[:, c1 * H:seg],
                                edges_ap[:, t0 + c1:t0 + caps[w], :].rearrange(
                                    "p t h -> p (t h)"))
                    else:
                        nc.sync.dma_start(
                            etile[:, off:off + seg],
                            edges_ap[:, t0:t0 + caps[w], :].rearrange(
                                "p t h -> p (t h)"))
                    off += seg
            else:
                nc.sync.dma_start(
                    etile[:],
                    edges_ap[:, gt0:gt0 + gtiles, :].rearrange(
                        "p t h -> p (t h)"))

        for g in range(n_groups):
            wlist, gt0, gtiles = gmeta[g]
            nwin = len(wlist)
            etile = etiles[g]
            c0 = g * GROUP * WIN
            ncols = nwin * WIN
            xg = xT_t[:, c0:c0 + ncols]

            pwg = pw.tile([H, nwin * WIN], f32, tag="pw")
            for wi, w in enumerate(wlist):
                t0 = tstart[w] - gt0
                pslice = pwg[:, wi * WIN:(wi + 1) * WIN]
                kov = caps[w] - cap_id[w]
                Sb = None
                if kov:
                    Sb = spool.tile([128, kov * WIN], bf16, tag="S")
                    i0 = ov_start[w]
                    nc.vector.scalar_tensor_tensor(
                        out=Sb[:].rearrange("p (k q) -> p k q", k=kov),
                        in0=iota_t.unsqueeze(1).to_broadcast([128, kov, WIN]),
                        scalar=1.0,
                        in1=colrel_t[:, i0:i0 + kov].unsqueeze(2).to_broadcast(
                            [128, kov, WIN]),
                        op0=mybir.AluOpType.bypass,
                        op1=mybir.AluOpType.is_equal)
                for t in range(caps[w]):
                    lhsT = etile[:, (t0 + t) * H:(t0 + t + 1) * H]
                    if t < cap_id[w]:
                        rhs = ident_t
                    else:
                        ko = t - cap_id[w]
                        rhs = Sb[:, ko * WIN:(ko + 1) * WIN]
                    nc.tensor.matmul(out=pslice, lhsT=lhsT, rhs=rhs,
                                     start=(t == 0), stop=(t == caps[w] - 1))
            nc.vector.tensor_copy(aggT[:, c0:c0 + ncols], pwg[:])

            phh = ph.tile([H, ncols], f32, tag="ph")
            nc.tensor.matmul(phh[:], lhsT=w1a_t, rhs=xg,
                             start=True, stop=False)
            nc.tensor.matmul(phh[:], lhsT=w1b_t, rhs=aggT[:, c0:c0 + ncols],
                             start=False, stop=True)
            hT = hpool.tile([H, ncols], bf16, tag="hT")
            nc.scalar.activation(out=hT[:], in_=phh[:],
                                 func=mybir.ActivationFunctionType.Relu,
                                 bias=b1_t, scale=1.0)
            if not c_zero:
                # per-feature b2 fold: hT += c (c = W2^-T b2, per partition)
                nc.scalar.add(hT[:], hT[:], c_t)
            pog = po.tile([128, ncols], f32, tag="po")
            for k in range(nwin):
                nc.tensor.matmul(pog[:, k * WIN:(k + 1) * WIN],
                                 lhsT=hT[:, k * WIN:(k + 1) * WIN],
                                 rhs=w2_t, start=(k == 0),
                                 stop=(k == nwin - 1))
            osb = opool.tile([128, ncols], out_dt, tag="osb")
            nc.scalar.copy(osb[:], pog[:])

            rows = min(NPC - c0, ncols)
            if rows == ncols:
                nc.gpsimd.dma_start(
                    out_ap[c0:c0 + ncols, :].rearrange("(t p) h -> p t h", p=128),
                    osb[:].rearrange("p (t h) -> p t h", t=nwin))
            else:
                full = rows // WIN
                for k in range(full):
                    nc.gpsimd.dma_start(out_ap[c0 + k * WIN:c0 + (k + 1) * WIN, :],
                                        osb[:, k * WIN:(k + 1) * WIN])
                rem = rows - full * WIN
                if rem:
                    nc.gpsimd.dma_start(
                        out_ap[c0 + full * WIN:c0 + rows, :],
                        osb[:rem, full * WIN:(full + 1) * WIN])

    nc.finalize()
    return nc


def kernel(x, edge_index, edge_attr, u=None, batch=None, W1=None, b1=None,
           W2=None, b2=None, **_unused):
    _install_axon_trace_shim()
    from concourse import mybir
    from concourse.bass_utils import run_bass_kernel_spmd

    np_edt, e_dt = ml_dtypes.float8_e3m4, mybir.dt.float8e3

    in_maps, cap_id, caps, tstart, T, NOV, c_zero = _prep_host(
        x, edge_index, edge_attr, W1, b1, W2, b2, np_edt)
    nc = _build_program(cap_id, caps, tstart, T, NOV, c_zero, e_dt)
    res = run_bass_kernel_spmd(nc, in_maps, core_ids=list(range(N_CORES)))
    out = np.concatenate(
        [np.asarray(res.results[c]["out"], np.float32) for c in range(N_CORES)],
        axis=0)
    return np.ascontiguousarray(out, dtype=np.float32)
